# revision 21
# baseline (speedup 1.0000x reference)
"""Trainium2 Bass kernel for the hierarchical 6-edge-type GNN (nn_AutoregressiveModel).

Strategy
--------
Scatter-add over a fixed edge list == multiply by a 0/1 adjacency matrix.
Node ids are level-ordered (level z = nodes [2^(z-1), 2^z)), and every edge
type connects level z_tgt to z_src with z_tgt - z_src in {0,1,2}.  At
128x128 tile granularity each type's adjacency has EXACTLY one nonzero
source chunk per target chunk:

    child/niephew:  src_chunk = tgt_chunk // 2
    grandchild:     src_chunk = tgt_chunk // 4
    self/sibling/cousin: src_chunk = tgt_chunk

So the whole message-passing layer is 8 PSUM-accumulated [128,128]x[128,512]
matmuls per edge type.  Per-type Linear commutes with gather (N << E), and
LayerNorm's g/beta fold into the next layer's weights (W' = diag(g) W,
b' = beta W + b).  The per-type bias scatter-adds to  indegree_t(n) * b'_t
== one rank-6 matmul per target chunk.

Everything on the PE runs in bf16 (adjacency tiles and degree vectors are
small integers -> exact; weights/activations round at ~4e-3 relative).
bf16 streams 1 col/cycle through the PE (fp32 is a 2-instruction LOW_HIGH
decomposition at ~4 cyc/col, f32r still ~2 cyc/col measured) and rides the
16-bit DMA-XBAR transpose path, so the inter-layer h -> hT transposes run
on the (otherwise idle) DMA queues instead of PE+DVE.

Layout: node-major (nodes on partitions).  Per layer, in lockstep over
chunks r (A-bank r only needs Y-chunks <= r, keeping PSUM live <= 8 banks):
  h --DMA-XBAR--> hT --(hT stationary x Wcat moving)--> Y (PSUM, f32)
    --DVE/ACT cast-copy--> Y SBUF bf16 --(A^T tiles stationary, Y moving
  [src, b*f])--> out accumulated in PSUM f32 --bn_stats/aggr (exact f32
  stats straight from PSUM); Sqrt/reciprocal/Tanh batched once per layer
  (ACT table loads cost 1.3us each) --> next h bf16 in SBUF.

Data-parallel over batch: 32 batches -> 4 per NeuronCore, 8 cores, params
and adjacency replicated, no collectives.
"""

import sys
import numpy as np

sys.path.insert(0, '/opt/trn_rl_repo')

import ml_dtypes                        # noqa: E402
import concourse.bass as bass          # noqa: E402,F401
import concourse.bacc as bacc          # noqa: E402
import concourse.mybir as mybir        # noqa: E402
import concourse.tile as tile          # noqa: E402
from concourse.bass_utils import run_bass_kernel_spmd   # noqa: E402

F32 = mybir.dt.float32
BF16 = mybir.dt.bfloat16
NPBF = ml_dtypes.bfloat16
LN_EPS = 1e-5
N_NODES = 1024
P = 128
NCHUNK = 8          # 1024 / 128
B_CORE = 4          # 32 batches / 8 cores
N_CORES = 8
TYPE_ORDER = ('child', 'sibling', 'niephew', 'cousin', 'grandchild', 'self')
CMAP = {
    'child': lambda r: r // 2,
    'niephew': lambda r: r // 2,
    'grandchild': lambda r: r // 4,
    'self': lambda r: r,
    'sibling': lambda r: r,
    'cousin': lambda r: r,
}
KPAD = 32           # pad tiny contraction dims (2, 5, 6) up to 32


def build_program():
    nc = bacc.Bacc('TRN2', target_bir_lowering=False, debug=False)

    # ---- DRAM I/O (all host-predigested, bf16) ----
    d_xT = nc.dram_tensor('xT', [KPAD, B_CORE * N_NODES], BF16, kind='ExternalInput')
    d_w1 = nc.dram_tensor('w1', [KPAD, 6 * 128], BF16, kind='ExternalInput')
    d_w2 = nc.dram_tensor('w2', [128, 6 * 128], BF16, kind='ExternalInput')
    d_w3 = nc.dram_tensor('w3', [128, 6 * 2], BF16, kind='ExternalInput')
    d_bcf1 = nc.dram_tensor('bcf1', [128, 6 * 128], BF16, kind='ExternalInput')
    d_bcf2 = nc.dram_tensor('bcf2', [128, 6 * 128], BF16, kind='ExternalInput')
    d_bcf3 = nc.dram_tensor('bcf3', [128, 6 * 2], BF16, kind='ExternalInput')
    d_at = nc.dram_tensor('atiles', [128, 48 * 128], BF16, kind='ExternalInput')
    d_out = nc.dram_tensor('out', [P, NCHUNK, B_CORE, 2], F32, kind='ExternalOutput')

    with tile.TileContext(nc) as tc:
        import contextlib
        with contextlib.ExitStack() as ctx:
            singles = ctx.enter_context(tc.tile_pool(name='singles', bufs=1))
            pk_hT = ctx.enter_context(tc.tile_pool(name='hT', bufs=6))
            pk_stat = ctx.enter_context(tc.tile_pool(name='stat', bufs=8))
            ps_w = ctx.enter_context(tc.tile_pool(name='psw', bufs=2, space='PSUM'))
            ps_tr = ctx.enter_context(tc.tile_pool(name='pstr', bufs=1, space='PSUM'))
            ps_out = ctx.enter_context(tc.tile_pool(name='psout', bufs=3, space='PSUM'))

            # ---- resident SBUF state ----
            sb_w1 = singles.tile([KPAD, 6 * 128], BF16)
            sb_w2 = singles.tile([128, 6 * 128], BF16)
            sb_w3 = singles.tile([128, 6 * 2], BF16)
            sb_bcf1 = singles.tile([128, 6 * 128], BF16)
            sb_bcf2 = singles.tile([128, 6 * 128], BF16)
            sb_bcf3 = singles.tile([128, 6 * 2], BF16)
            sb_id = singles.tile([128, 128], BF16)
            sb_at = singles.tile([128, 48, 128], BF16)
            sb_eps = singles.tile([128, 1], F32)
            sb_y = singles.tile([128, NCHUNK, 6, B_CORE, 128], BF16)
            sb_y3 = singles.tile([128, NCHUNK, 6, B_CORE, 2], BF16)
            sb_out = singles.tile([128, NCHUNK, B_CORE, 2], F32)
            sb_mv1 = singles.tile([128, NCHUNK, B_CORE, 2], F32)
            sb_mv2 = singles.tile([128, NCHUNK, B_CORE, 2], F32)
            sb_ln1 = singles.tile([128, 4, NCHUNK * B_CORE], F32)
            sb_ln2 = singles.tile([128, 4, NCHUNK * B_CORE], F32)
            sb_h2 = singles.tile([128, NCHUNK, B_CORE, 128], BF16)
            sb_h3 = singles.tile([128, NCHUNK, B_CORE, 128], BF16)
            sb_xT = singles.tile([KPAD, B_CORE * N_NODES], BF16)

            nc.sync.dma_start(out=sb_xT[:], in_=d_xT[:])
            nc.sync.dma_start(out=sb_w1[:], in_=d_w1[:])
            nc.sync.dma_start(out=sb_w2[:], in_=d_w2[:])
            nc.sync.dma_start(out=sb_w3[:], in_=d_w3[:])
            # adjacency tiles first (L1's A-phase needs them early);
            # contiguous col-slices, three queues
            at_f = sb_at[:].rearrange('p k t -> p (k t)')
            nc.scalar.dma_start(out=at_f[:, 0:2048], in_=d_at[:, 0:2048])
            nc.gpsimd.dma_start(out=at_f[:, 2048:4096], in_=d_at[:, 2048:4096])
            nc.gpsimd.dma_start(out=at_f[:, 4096:6144], in_=d_at[:, 4096:6144])
            nc.scalar.dma_start(out=sb_bcf1[:], in_=d_bcf1[:])
            nc.scalar.dma_start(out=sb_bcf2[:], in_=d_bcf2[:])
            nc.scalar.dma_start(out=sb_bcf3[:], in_=d_bcf3[:])
            nc.vector.memset(sb_eps[:], LN_EPS)
            from concourse.masks import make_identity
            make_identity(nc, sb_id[:])

            # HAM warm-up: ~20 dummy matmuls while the DMA loads run, so the
            # PE clock-gate reaches 8/8 before real work starts
            wu_l = singles.tile([128, 128], BF16)
            wu_r = singles.tile([128, 512], BF16)
            nc.vector.memset(wu_l[:], 0.0)
            nc.vector.memset(wu_r[:], 0.0)
            pwu = ps_w.tile([128, 768], F32, tag='w12')
            for _ in range(20):
                nc.tensor.matmul(pwu[:, 0:512], wu_l[:], wu_r[:],
                                 start=True, stop=True)

            def w_chunk(layer, c):
                """Produce Y[:, c, :, :, :] for one node chunk (all batches)."""
                for b in range(B_CORE):
                    if layer == 1:
                        lhsT = sb_xT[:, b * N_NODES + c * 128:
                                     b * N_NODES + (c + 1) * 128]
                    elif layer == 2:
                        hT = pk_hT.tile([128, 128], BF16, tag='hT')
                        eng = nc.sync if b % 2 == 0 else nc.scalar
                        eng.dma_start(out=hT[:], in_=sb_h2[:, c, b, :],
                                      transpose=True)
                        lhsT = hT[:]
                    else:
                        pt = ps_tr.tile([128, 128], BF16, tag='tr')
                        nc.tensor.transpose(pt[:], sb_h3[:, c, b, :], sb_id[:])
                        hT = pk_hT.tile([128, 128], BF16, tag='hT')
                        nc.vector.tensor_copy(hT[:], pt[:])
                        lhsT = hT[:]
                    pw = ps_w.tile([128, 768], F32, tag='w12')
                    if layer == 3:
                        nc.tensor.matmul(pw[:, 0:12], lhsT, sb_w3[:],
                                         start=True, stop=True)
                        nc.vector.scalar_tensor_tensor(
                            sb_y3[:, c, :, b, :], pw[:, 0:12], 1.0, sb_bcf3[:],
                            op0=mybir.AluOpType.mult, op1=mybir.AluOpType.add)
                        continue
                    sw = sb_w1 if layer == 1 else sb_w2
                    wid = 640 if layer == 1 else 768
                    nc.tensor.matmul(pw[:, 0:512], lhsT, sw[:, 0:512],
                                     start=True, stop=True)
                    nc.tensor.matmul(pw[:, 512:wid], lhsT, sw[:, 512:wid],
                                     start=True, stop=True)
                    sb_bcf = sb_bcf1 if layer == 1 else sb_bcf2
                    ntw = wid // 128
                    nc.vector.scalar_tensor_tensor(
                        sb_y[:, c, 0:ntw, b, :],
                        pw[:, 0:wid], 1.0, sb_bcf[:, 0:wid],
                        op0=mybir.AluOpType.mult, op1=mybir.AluOpType.add)

            def a_bank(layer, r, harg):
                """Accumulate out bank r over bias + all edge types, then stats."""
                ntyp = 5 if layer == 1 else 6
                ncol = 8 if layer == 3 else 512
                yy = sb_y3 if layer == 3 else sb_y
                po = ps_out.tile([128, 512], F32, tag='out')
                for ti in range(ntyp):
                    typ = TYPE_ORDER[ti]
                    c = CMAP[typ](r)
                    nc.tensor.matmul(po[:, 0:ncol], sb_at[:, ti * 8 + r, :],
                                     yy[:, c, ti, :, :],
                                     start=(ti == 0), stop=(ti == ntyp - 1))
                if layer == 3:
                    nc.vector.tensor_copy(sb_out[:, r, :, :], po[:, 0:8])
                    return
                # raw (pre-norm) copy out of PSUM; stats inline; tanh batched
                sb_h_next, sb_mv = harg
                nc.vector.tensor_copy(sb_h_next[:, r, :, :], po[:, 0:512])
                st4 = pk_stat.tile([128, 4, 6], F32, tag='st')
                for b in range(B_CORE):
                    nc.vector.bn_stats(st4[:, b, :], po[:, b * 128:(b + 1) * 128])
                    nc.vector.bn_aggr(sb_mv[:, r, b, :], st4[:, b, :])

            def ln_tanh_batch(sb_h, sb_mv, sb_ln, half):
                # batched rstd math + 16 in-place Tanh activations
                r0, r1 = (0, 4) if half == 0 else (4, 8)
                i0, i1 = r0 * B_CORE, r1 * B_CORE
                var = sb_mv[:].rearrange('p r b s -> p (r b) s')[:, i0:i1, 1]
                mu = sb_mv[:].rearrange('p r b s -> p (r b) s')[:, i0:i1, 0]
                sd = sb_ln[:, 0, i0:i1]
                rstd = sb_ln[:, 1, i0:i1]
                nmr = sb_ln[:, 2, i0:i1]
                nc.scalar.activation(sd, var,
                                     mybir.ActivationFunctionType.Sqrt,
                                     bias=sb_eps[:], scale=1.0)
                nc.vector.reciprocal(rstd, sd)
                nc.vector.scalar_tensor_tensor(
                    nmr, mu, -1.0, rstd,
                    op0=mybir.AluOpType.mult, op1=mybir.AluOpType.mult)
                for r in range(r0, r1):
                    for b in range(B_CORE):
                        i = r * B_CORE + b
                        nc.scalar.activation(sb_h[:, r, b, :], sb_h[:, r, b, :],
                                             mybir.ActivationFunctionType.Tanh,
                                             bias=sb_ln[:, 2, i:i + 1],
                                             scale=sb_ln[:, 1, i:i + 1])

            for r in range(NCHUNK):
                w_chunk(1, r)
                a_bank(1, r, (sb_h2, sb_mv1))
                if r in (3, 7):
                    ln_tanh_batch(sb_h2, sb_mv1, sb_ln1, 0 if r == 3 else 1)

            for r in range(NCHUNK):
                w_chunk(2, r)
                a_bank(2, r, (sb_h3, sb_mv2))
                if r in (3, 7):
                    ln_tanh_batch(sb_h3, sb_mv2, sb_ln2, 0 if r == 3 else 1)

            for r in range(NCHUNK):
                w_chunk(3, r)
                a_bank(3, r, None)

            nc.sync.dma_start(out=d_out[:], in_=sb_out[:])

    nc.compile()
    return nc


def prep_host(x, params, edges):
    """Digest full inputs into per-core DRAM arrays (numpy only)."""
    x = np.asarray(x, np.float32)                      # [32, 1024, 2]
    gc = params['gc']
    ln = params['ln']

    def fold(layer):
        Wd, bd = {}, {}
        for t in TYPE_ORDER:
            W = np.asarray(gc[layer][t]['W'], np.float32)
            b = np.asarray(gc[layer][t]['b'], np.float32)
            if layer == 0:
                Wd[t], bd[t] = W, b
            else:
                g = np.asarray(ln[layer - 1]['g'], np.float32)
                beta = np.asarray(ln[layer - 1]['b'], np.float32)
                Wd[t] = g[:, None] * W
                bd[t] = beta @ W + b
        return Wd, bd

    W1, b1 = fold(0)
    W2, b2 = fold(1)
    W3, b3 = fold(2)

    w1 = np.zeros((KPAD, 6 * 128), np.float32)
    for ti in range(5):
        w1[0:2, ti * 128:(ti + 1) * 128] = W1[TYPE_ORDER[ti]]
    w2 = np.zeros((128, 6 * 128), np.float32)
    w3 = np.zeros((128, 6 * 2), np.float32)
    for ti, t in enumerate(TYPE_ORDER):
        w2[:, ti * 128:(ti + 1) * 128] = W2[t]
        w3[:, ti * 2:(ti + 1) * 2] = W3[t]

    atiles = np.zeros((48, 128, 128), np.float32)
    for ti, t in enumerate(TYPE_ORDER):
        s, tt = edges[t]
        s = np.asarray(s).astype(np.int64)
        tt = np.asarray(tt).astype(np.int64)
        r = tt // 128
        cexp = np.array([CMAP[t](ri) for ri in range(8)])[r]
        assert np.all(s // 128 == cexp), f'tile map violated for {t}'
        atiles[ti * 8 + r, s % 128, tt % 128] = 1.0

    # per-type bias rows, broadcast across partitions, added into Y
    # (out = sum_t A_t (Y_t + b'_t)  reproduces the deg_t * b'_t field)
    bcf1 = np.zeros((128, 6 * 128), np.float32)
    bcf2 = np.zeros((128, 6 * 128), np.float32)
    bcf3 = np.zeros((128, 6 * 2), np.float32)
    for ti, t in enumerate(TYPE_ORDER):
        if ti < 5:
            bcf1[:, ti * 128:(ti + 1) * 128] = b1[t][None, :]
        bcf2[:, ti * 128:(ti + 1) * 128] = b2[t][None, :]
        bcf3[:, ti * 2:(ti + 1) * 2] = b3[t][None, :]
    atiles_t = atiles.transpose(1, 0, 2).reshape(128, 48 * 128)
    shared = {k: v.astype(NPBF) for k, v in
              dict(w1=w1, w2=w2, w3=w3, bcf1=bcf1, bcf2=bcf2, bcf3=bcf3,
                   atiles=atiles_t).items()}
    xTs = []
    for i in range(N_CORES):
        xc = x[i * B_CORE:(i + 1) * B_CORE]            # [4, 1024, 2]
        xT = np.zeros((KPAD, B_CORE * N_NODES), np.float32)
        xT[0:2] = xc.transpose(2, 0, 1).reshape(2, B_CORE * N_NODES)
        xTs.append(xT.astype(NPBF))
    return shared, xTs


_NC_CACHE = []


def _get_nc():
    if not _NC_CACHE:
        _NC_CACHE.append(build_program())
    return _NC_CACHE[0]


def _run(x, params, edges, trace=False, tmpdir=None):
    shared, xTs = prep_host(x, params, edges)
    nc = _get_nc()
    in_maps = [dict(shared, xT=xTs[i]) for i in range(N_CORES)]
    res = run_bass_kernel_spmd(nc, in_maps, list(range(N_CORES)),
                               trace=trace, tmpdir=tmpdir)
    full = np.zeros((32, N_NODES, 2), np.float32)
    for i in range(N_CORES):
        o = res.results[i]['out']                      # [128, 8, 4, 2]
        full[i * B_CORE:(i + 1) * B_CORE] = (
            o.transpose(2, 1, 0, 3).reshape(B_CORE, N_NODES, 2))
    return full, res


def kernel(x, params, edges):
    full, _ = _run(x, params, edges)
    return full


# revision 27
# speedup vs baseline: 1.0787x; 1.0787x over previous
"""Trainium2 Bass kernel for the hierarchical 6-edge-type GNN (nn_AutoregressiveModel).

Strategy
--------
Scatter-add over a fixed edge list == multiply by a 0/1 adjacency matrix.
Node ids are level-ordered (level z = nodes [2^(z-1), 2^z)), and every edge
type connects level z_tgt to z_src with z_tgt - z_src in {0,1,2}.  At
128x128 tile granularity each type's adjacency has EXACTLY one nonzero
source chunk per target chunk:

    child/niephew:  src_chunk = tgt_chunk // 2
    grandchild:     src_chunk = tgt_chunk // 4
    self/sibling/cousin: src_chunk = tgt_chunk

So the whole message-passing layer is 8 PSUM-accumulated [128,128]x[128,512]
matmuls per edge type.  Per-type Linear commutes with gather (N << E), and
LayerNorm's g/beta fold into the next layer's weights (W' = diag(g) W,
b' = beta W + b).  The per-type bias scatter-adds to  indegree_t(n) * b'_t
== one rank-6 matmul per target chunk.

Everything on the PE runs in bf16 (adjacency tiles and degree vectors are
small integers -> exact; weights/activations round at ~4e-3 relative).
bf16 streams 1 col/cycle through the PE (fp32 is a 2-instruction LOW_HIGH
decomposition at ~4 cyc/col, f32r still ~2 cyc/col measured) and rides the
16-bit DMA-XBAR transpose path, so the inter-layer h -> hT transposes run
on the (otherwise idle) DMA queues instead of PE+DVE.

Layout: node-major (nodes on partitions).  Per layer, in lockstep over
chunks r (A-bank r only needs Y-chunks <= r, keeping PSUM live <= 8 banks):
  h --DMA-XBAR--> hT --(hT stationary x Wcat moving)--> Y (PSUM, f32)
    --DVE/ACT cast-copy--> Y SBUF bf16 --(A^T tiles stationary, Y moving
  [src, b*f])--> out accumulated in PSUM f32 --bn_stats/aggr (exact f32
  stats straight from PSUM); Sqrt/reciprocal/Tanh batched once per layer
  (ACT table loads cost 1.3us each) --> next h bf16 in SBUF.

Data-parallel over batch: 32 batches -> 4 per NeuronCore, 8 cores, params
and adjacency replicated, no collectives.
"""

import sys
import numpy as np

sys.path.insert(0, '/opt/trn_rl_repo')

import ml_dtypes                        # noqa: E402
import concourse.bass as bass          # noqa: E402,F401
import concourse.bacc as bacc          # noqa: E402
import concourse.mybir as mybir        # noqa: E402
import concourse.tile as tile          # noqa: E402
from concourse.bass_utils import run_bass_kernel_spmd   # noqa: E402

F32 = mybir.dt.float32
BF16 = mybir.dt.bfloat16
NPBF = ml_dtypes.bfloat16
LN_EPS = 1e-5
N_NODES = 1024
P = 128
NCHUNK = 8          # 1024 / 128
B_CORE = 4          # 32 batches / 8 cores
N_CORES = 8
TYPE_ORDER = ('child', 'sibling', 'niephew', 'cousin', 'grandchild', 'self')
CMAP = {
    'child': lambda r: r // 2,
    'niephew': lambda r: r // 2,
    'grandchild': lambda r: r // 4,
    'self': lambda r: r,
    'sibling': lambda r: r,
    'cousin': lambda r: r,
}
KPAD = 32           # pad tiny contraction dims (2, 5, 6) up to 32


def build_program():
    nc = bacc.Bacc('TRN2', target_bir_lowering=False, debug=False)

    # ---- DRAM I/O (all host-predigested, bf16) ----
    d_xT = nc.dram_tensor('xT', [KPAD, B_CORE * N_NODES], BF16, kind='ExternalInput')
    d_w1 = nc.dram_tensor('w1', [KPAD, 6 * 128], BF16, kind='ExternalInput')
    d_w2 = nc.dram_tensor('w2', [128, 6 * 128], BF16, kind='ExternalInput')
    d_w3 = nc.dram_tensor('w3', [128, 6 * 2], BF16, kind='ExternalInput')
    d_bcf1 = nc.dram_tensor('bcf1', [128, 6 * 128], BF16, kind='ExternalInput')
    d_bcf2 = nc.dram_tensor('bcf2', [128, 6 * 128], BF16, kind='ExternalInput')
    d_bcf3 = nc.dram_tensor('bcf3', [128, 6 * 2], BF16, kind='ExternalInput')
    d_at = nc.dram_tensor('atiles', [128, 48 * 128], BF16, kind='ExternalInput')
    d_out = nc.dram_tensor('out', [P, NCHUNK, B_CORE, 2], F32, kind='ExternalOutput')

    with tile.TileContext(nc) as tc:
        import contextlib
        with contextlib.ExitStack() as ctx:
            singles = ctx.enter_context(tc.tile_pool(name='singles', bufs=1))
            pk_hT = ctx.enter_context(tc.tile_pool(name='hT', bufs=6))
            pk_stat = ctx.enter_context(tc.tile_pool(name='stat', bufs=8))
            ps_w = ctx.enter_context(tc.tile_pool(name='psw', bufs=2, space='PSUM'))
            ps_tr = ctx.enter_context(tc.tile_pool(name='pstr', bufs=2, space='PSUM'))
            ps_out = ctx.enter_context(tc.tile_pool(name='psout', bufs=2, space='PSUM'))

            # ---- resident SBUF state ----
            sb_w1 = singles.tile([KPAD, 6 * 128], BF16)
            sb_w2 = singles.tile([128, 6 * 128], BF16)
            sb_w3 = singles.tile([128, 6 * 2], BF16)
            sb_bcf1 = singles.tile([128, 6 * 128], BF16)
            sb_bcf2 = singles.tile([128, 6 * 128], BF16)
            sb_bcf3 = singles.tile([128, 6 * 2], BF16)
            sb_id = singles.tile([128, 128], BF16)
            sb_at = singles.tile([128, 48, 128], BF16)
            sb_eps = singles.tile([128, 1], F32)
            sb_y = singles.tile([128, NCHUNK, 6, B_CORE, 128], BF16)
            sb_y3 = singles.tile([128, NCHUNK, 6, B_CORE, 2], BF16)
            sb_out = singles.tile([128, NCHUNK, B_CORE, 2], F32)
            sb_mv1 = singles.tile([128, NCHUNK, B_CORE, 2], F32)
            sb_mv2 = singles.tile([128, NCHUNK, B_CORE, 2], F32)
            sb_ln1 = singles.tile([128, 4, NCHUNK * B_CORE], F32)
            sb_ln2 = singles.tile([128, 4, NCHUNK * B_CORE], F32)
            sb_h2 = singles.tile([128, NCHUNK, B_CORE, 128], BF16)
            sb_h3 = singles.tile([128, NCHUNK, B_CORE, 128], BF16)
            sb_xT = singles.tile([KPAD, B_CORE * N_NODES], BF16)

            nc.sync.dma_start(out=sb_xT[:], in_=d_xT[:])
            nc.sync.dma_start(out=sb_w1[:], in_=d_w1[:])
            nc.sync.dma_start(out=sb_w2[:], in_=d_w2[:])
            nc.sync.dma_start(out=sb_w3[:], in_=d_w3[:])
            # adjacency tiles first (L1's A-phase needs them early);
            # contiguous col-slices, three queues
            at_f = sb_at[:].rearrange('p k t -> p (k t)')
            nc.scalar.dma_start(out=at_f[:, 0:2048], in_=d_at[:, 0:2048])
            nc.gpsimd.dma_start(out=at_f[:, 2048:4096], in_=d_at[:, 2048:4096])
            nc.gpsimd.dma_start(out=at_f[:, 4096:6144], in_=d_at[:, 4096:6144])
            nc.scalar.dma_start(out=sb_bcf1[:], in_=d_bcf1[:])
            nc.scalar.dma_start(out=sb_bcf2[:], in_=d_bcf2[:])
            nc.scalar.dma_start(out=sb_bcf3[:], in_=d_bcf3[:])
            nc.vector.memset(sb_eps[:], LN_EPS)
            from concourse.masks import make_identity
            make_identity(nc, sb_id[:])

            # HAM warm-up: ~20 dummy matmuls while the DMA loads run, so the
            # PE clock-gate reaches 8/8 before real work starts
            wu_l = singles.tile([128, 128], BF16)
            wu_r = singles.tile([128, 512], BF16)
            nc.vector.memset(wu_l[:], 0.0)
            nc.vector.memset(wu_r[:], 0.0)
            pwu = ps_w.tile([128, 768], F32, tag='w12')
            for _ in range(20):
                nc.tensor.matmul(pwu[:, 0:512], wu_l[:], wu_r[:],
                                 start=True, stop=True)

            def w_chunk(layer, c):
                """Produce Y[:, c, :, :, :] for one node chunk (all batches)."""
                for b in range(B_CORE):
                    if layer == 1:
                        lhsT = sb_xT[:, b * N_NODES + c * 128:
                                     b * N_NODES + (c + 1) * 128]
                    elif layer == 2:
                        hT = pk_hT.tile([128, 128], BF16, tag='hT')
                        eng = nc.sync if b % 2 == 0 else nc.scalar
                        eng.dma_start(out=hT[:], in_=sb_h2[:, c, b, :],
                                      transpose=True)
                        lhsT = hT[:]
                    else:
                        pt = ps_tr.tile([128, 128], BF16, tag='tr')
                        nc.tensor.transpose(pt[:], sb_h3[:, c, b, :], sb_id[:])
                        hT = pk_hT.tile([128, 128], BF16, tag='hT')
                        nc.vector.tensor_copy(hT[:], pt[:])
                        lhsT = hT[:]
                    pw = ps_w.tile([128, 768], F32, tag='w12')
                    if layer == 3:
                        nc.tensor.matmul(pw[:, 0:12], lhsT, sb_w3[:],
                                         start=True, stop=True)
                        nc.vector.scalar_tensor_tensor(
                            sb_y3[:, c, :, b, :], pw[:, 0:12], 1.0, sb_bcf3[:],
                            op0=mybir.AluOpType.mult, op1=mybir.AluOpType.add)
                        continue
                    sw = sb_w1 if layer == 1 else sb_w2
                    wid = 640 if layer == 1 else 768
                    nc.tensor.matmul(pw[:, 0:512], lhsT, sw[:, 0:512],
                                     start=True, stop=True)
                    nc.tensor.matmul(pw[:, 512:wid], lhsT, sw[:, 512:wid],
                                     start=True, stop=True)
                    sb_bcf = sb_bcf1 if layer == 1 else sb_bcf2
                    ntw = wid // 128
                    nc.vector.scalar_tensor_tensor(
                        sb_y[:, c, 0:ntw, b, :],
                        pw[:, 0:wid], 1.0, sb_bcf[:, 0:wid],
                        op0=mybir.AluOpType.mult, op1=mybir.AluOpType.add)

            def a_bank(layer, r, harg):
                """Accumulate out bank r over bias + all edge types, then stats."""
                ntyp = 5 if layer == 1 else 6
                ncol = 8 if layer == 3 else 512
                yy = sb_y3 if layer == 3 else sb_y
                po = ps_out.tile([128, 512], F32, tag='out')
                for ti in range(ntyp):
                    typ = TYPE_ORDER[ti]
                    c = CMAP[typ](r)
                    nc.tensor.matmul(po[:, 0:ncol], sb_at[:, ti * 8 + r, :],
                                     yy[:, c, ti, :, :],
                                     start=(ti == 0), stop=(ti == ntyp - 1))
                if layer == 3:
                    nc.vector.tensor_copy(sb_out[:, r, :, :], po[:, 0:8])
                    return
                # raw (pre-norm) copy out of PSUM; stats inline; tanh batched
                sb_h_next, sb_mv = harg
                nc.vector.tensor_copy(sb_h_next[:, r, :, :], po[:, 0:512])
                st4 = pk_stat.tile([128, 4, 6], F32, tag='st')
                for b in range(B_CORE):
                    nc.vector.bn_stats(st4[:, b, :], po[:, b * 128:(b + 1) * 128])
                    nc.vector.bn_aggr(sb_mv[:, r, b, :], st4[:, b, :])

            def ln_tanh_batch(sb_h, sb_mv, sb_ln, half):
                # batched rstd math + 16 in-place Tanh activations
                r0, r1 = (0, 4) if half == 0 else (4, 8)
                i0, i1 = r0 * B_CORE, r1 * B_CORE
                var = sb_mv[:].rearrange('p r b s -> p (r b) s')[:, i0:i1, 1]
                mu = sb_mv[:].rearrange('p r b s -> p (r b) s')[:, i0:i1, 0]
                sd = sb_ln[:, 0, i0:i1]
                rstd = sb_ln[:, 1, i0:i1]
                nmr = sb_ln[:, 2, i0:i1]
                nc.scalar.activation(sd, var,
                                     mybir.ActivationFunctionType.Sqrt,
                                     bias=sb_eps[:], scale=1.0)
                nc.vector.reciprocal(rstd, sd)
                nc.vector.scalar_tensor_tensor(
                    nmr, mu, -1.0, rstd,
                    op0=mybir.AluOpType.mult, op1=mybir.AluOpType.mult)
                for r in range(r0, r1):
                    for b in range(B_CORE):
                        i = r * B_CORE + b
                        nc.scalar.activation(sb_h[:, r, b, :], sb_h[:, r, b, :],
                                             mybir.ActivationFunctionType.Tanh,
                                             bias=sb_ln[:, 2, i:i + 1],
                                             scale=sb_ln[:, 1, i:i + 1])

            for r in range(NCHUNK):
                w_chunk(1, r)
                a_bank(1, r, (sb_h2, sb_mv1))
                if r in (3, 7):
                    ln_tanh_batch(sb_h2, sb_mv1, sb_ln1, 0 if r == 3 else 1)

            for r in range(NCHUNK):
                w_chunk(2, r)
                a_bank(2, r, (sb_h3, sb_mv2))
                if r in (3, 7):
                    ln_tanh_batch(sb_h3, sb_mv2, sb_ln2, 0 if r == 3 else 1)

            for r in range(NCHUNK):
                w_chunk(3, r)
                a_bank(3, r, None)

            nc.sync.dma_start(out=d_out[:], in_=sb_out[:])

    nc.compile()
    return nc


def prep_host(x, params, edges):
    """Digest full inputs into per-core DRAM arrays (numpy only)."""
    x = np.asarray(x, np.float32)                      # [32, 1024, 2]
    gc = params['gc']
    ln = params['ln']

    def fold(layer):
        Wd, bd = {}, {}
        for t in TYPE_ORDER:
            W = np.asarray(gc[layer][t]['W'], np.float32)
            b = np.asarray(gc[layer][t]['b'], np.float32)
            if layer == 0:
                Wd[t], bd[t] = W, b
            else:
                g = np.asarray(ln[layer - 1]['g'], np.float32)
                beta = np.asarray(ln[layer - 1]['b'], np.float32)
                Wd[t] = g[:, None] * W
                bd[t] = beta @ W + b
        return Wd, bd

    W1, b1 = fold(0)
    W2, b2 = fold(1)
    W3, b3 = fold(2)

    w1 = np.zeros((KPAD, 6 * 128), np.float32)
    for ti in range(5):
        w1[0:2, ti * 128:(ti + 1) * 128] = W1[TYPE_ORDER[ti]]
    w2 = np.zeros((128, 6 * 128), np.float32)
    w3 = np.zeros((128, 6 * 2), np.float32)
    for ti, t in enumerate(TYPE_ORDER):
        w2[:, ti * 128:(ti + 1) * 128] = W2[t]
        w3[:, ti * 2:(ti + 1) * 2] = W3[t]

    atiles = np.zeros((48, 128, 128), np.float32)
    for ti, t in enumerate(TYPE_ORDER):
        s, tt = edges[t]
        s = np.asarray(s).astype(np.int64)
        tt = np.asarray(tt).astype(np.int64)
        r = tt // 128
        cexp = np.array([CMAP[t](ri) for ri in range(8)])[r]
        assert np.all(s // 128 == cexp), f'tile map violated for {t}'
        atiles[ti * 8 + r, s % 128, tt % 128] = 1.0

    # per-type bias rows, broadcast across partitions, added into Y
    # (out = sum_t A_t (Y_t + b'_t)  reproduces the deg_t * b'_t field)
    bcf1 = np.zeros((128, 6 * 128), np.float32)
    bcf2 = np.zeros((128, 6 * 128), np.float32)
    bcf3 = np.zeros((128, 6 * 2), np.float32)
    for ti, t in enumerate(TYPE_ORDER):
        if ti < 5:
            bcf1[:, ti * 128:(ti + 1) * 128] = b1[t][None, :]
        bcf2[:, ti * 128:(ti + 1) * 128] = b2[t][None, :]
        bcf3[:, ti * 2:(ti + 1) * 2] = b3[t][None, :]
    atiles_t = atiles.transpose(1, 0, 2).reshape(128, 48 * 128)
    shared = {k: v.astype(NPBF) for k, v in
              dict(w1=w1, w2=w2, w3=w3, bcf1=bcf1, bcf2=bcf2, bcf3=bcf3,
                   atiles=atiles_t).items()}
    xTs = []
    for i in range(N_CORES):
        xc = x[i * B_CORE:(i + 1) * B_CORE]            # [4, 1024, 2]
        xT = np.zeros((KPAD, B_CORE * N_NODES), np.float32)
        xT[0:2] = xc.transpose(2, 0, 1).reshape(2, B_CORE * N_NODES)
        xTs.append(xT.astype(NPBF))
    return shared, xTs


_NC_CACHE = []


def _get_nc():
    if not _NC_CACHE:
        _NC_CACHE.append(build_program())
    return _NC_CACHE[0]


def _run(x, params, edges, trace=False, tmpdir=None):
    shared, xTs = prep_host(x, params, edges)
    nc = _get_nc()
    in_maps = [dict(shared, xT=xTs[i]) for i in range(N_CORES)]
    res = run_bass_kernel_spmd(nc, in_maps, list(range(N_CORES)),
                               trace=trace, tmpdir=tmpdir)
    full = np.zeros((32, N_NODES, 2), np.float32)
    for i in range(N_CORES):
        o = res.results[i]['out']                      # [128, 8, 4, 2]
        full[i * B_CORE:(i + 1) * B_CORE] = (
            o.transpose(2, 1, 0, 3).reshape(B_CORE, N_NODES, 2))
    return full, res


def kernel(x, params, edges):
    full, _ = _run(x, params, edges)
    return full
